# revision 1
# baseline (speedup 1.0000x reference)
"""Trainium2 Bass kernel for BlockAttnResLayer.

Computation (reference):
  V = concat([blocks, partial[None]])            # [9, B*T, D]
  rms = sqrt(mean(V^2, -1) + 1e-8)
  logits[n,t] = (V[n,t,:] . (norm_scale*proj_w)) / rms[n,t]
  alpha = softmax(logits, axis=n)
  h = sum_n alpha * V
  f = gelu(h @ W1) @ W2                          # tanh-approx gelu
  new_partial = partial + f
  returns (h, new_partial)

Sharding: pure data-parallel over tokens (B*T = 4096 -> 512/core on 8 cores).
Weights replicated; FFN matmuls run in float32r (fp32 with 11-bit mantissa,
1 cycle/row on the PE at N>=256 vs 4 cycles/row for plain fp32).

Overlap structure (overlap=True): attention emits token-tiles 0,1 first;
MM1 then runs over token-half A (N=256) for every f-chunk while attention
finishes tiles 2,3 on DVE/ACT, spilling gelu(half-a) to a DRAM scratch.
After attention, MM1 half-B re-streams W1 and MM2 consumes half-B act from
SBUF plus half-A act read back from DRAM, accumulating F quarter-groups
into an SBUF accumulator.  Costs ~96 MiB extra DMA, buys ~A/2 of PE overlap.
"""
import numpy as np
from contextlib import ExitStack

import concourse.bass as bass
import concourse.bacc as bacc
import concourse.tile as tile
from concourse import mybir
from concourse.bass_utils import run_bass_kernel_spmd
from concourse.masks import make_identity

f32 = mybir.dt.float32
f32r = mybir.dt.float32r
AF = mybir.ActivationFunctionType
ALU = mybir.AluOpType

N_CORES = 8
NB = 8            # completed blocks
N1 = 9            # blocks + partial
B, T, D, F = 2, 2048, 2048, 8192
TOK = B * T       # 4096
TPC = TOK // N_CORES  # 512 tokens per core
P = 128
TT = TPC // P     # 4 token tiles per core
TH = TPC // 2     # 256-token halves
DC = D // P       # 16 d-chunks
FC = F // P       # 64 f-chunks
NG = 4            # f-chunk quarter groups for MM2 accumulation
FG = FC // NG     # 16 f-chunks per group
NQ = D // 512     # 4 output column quarters
EPS = 1e-8


def round_f32r(x: np.ndarray) -> np.ndarray:
    """RNE-round fp32 to 11 explicit mantissa bits (the PE's fp32r format)."""
    v = x.astype(np.float32).view(np.uint32).astype(np.uint64)
    lsb = (v >> 12) & 1
    v = v + 0x7FF + lsb
    v = (v & np.uint64(0xFFFFF000)).astype(np.uint32)
    return v.view(np.float32)


def retile_w1(w1r: np.ndarray) -> np.ndarray:
    """[D, F] -> [FC, P, DC, P] with w1t[fc, p, kc, q] = W1[kc*P+p, fc*P+q]."""
    return np.ascontiguousarray(
        w1r.reshape(DC, P, FC, P).transpose(2, 1, 0, 3))


def build_nc(n_reps: int = 1, gelu: bool = True, phase_a: bool = True,
             phase_b: bool = True, overlap: bool = True):
    act_fn = AF.Gelu_apprx_tanh if gelu else AF.Copy
    nc = bacc.Bacc("TRN2", target_bir_lowering=False, debug=False, num_devices=N_CORES)
    vb = nc.dram_tensor("vb", [N1, TPC, D], f32, kind="ExternalInput").ap()
    # w1 host-retiled to [FC, P, DC, P]: w1t[fc, p, kc, q] = W1[kc*128+p, fc*128+q]
    # so each weight-tile DMA reads one contiguous 8KB run per partition.
    w1 = nc.dram_tensor("w1", [FC, P, DC, P], f32r, kind="ExternalInput").ap()
    w2 = nc.dram_tensor("w2", [F, D], f32r, kind="ExternalInput").ap()
    pjw = nc.dram_tensor("pjw", [D], f32, kind="ExternalInput").ap()
    nsw = nc.dram_tensor("nsw", [D], f32, kind="ExternalInput").ap()
    h_out = nc.dram_tensor("h_out", [TPC, D], f32, kind="ExternalOutput").ap()
    np_out = nc.dram_tensor("np_out", [TPC, D], f32, kind="ExternalOutput").ap()
    act_d = nc.dram_tensor("act_d", [FC, P, TH], f32r).ap()   # half-A act spill

    h_out_t = h_out.rearrange("(tt p) d -> tt p d", p=P)

    with tile.TileContext(nc) as tc, ExitStack() as ctx:
        outer = ctx.enter_context(tc.tile_pool(name="outer", bufs=1))
        pw_b = outer.tile([P, D], f32)
        # transposed h in two token-halves: hTs[half][k] is [P, TH]
        hTs = [[outer.tile([P, TH], f32r, name=f"hT{hf}_{k}") for k in range(DC)]
               for hf in range(2)]
        # FFN pools that must be live during attention for overlap
        w1p = ctx.enter_context(tc.tile_pool(name="w1p", bufs=2))
        ps1p = ctx.enter_context(tc.tile_pool(name="ps1p", bufs=2, space="PSUM"))
        aspp = ctx.enter_context(tc.tile_pool(name="aspp", bufs=6))

        def mm1_half(hf, fc, dst_ap, w1t=None):
            """One f-chunk of MM1 over token half hf -> gelu -> dst_ap (SBUF).

            Weight DMAs ride the ACT engine's HW queue so they never head-of-line
            block the V-tile loads on the SP queue."""
            if w1t is None:
                w1t = w1p.tile([P, DC, P], f32r, name="w1t")
                nc.scalar.dma_start(out=w1t, in_=w1[fc])
            ps1 = ps1p.tile([P, TH], f32, name="ps1")
            for k in range(DC):
                nc.tensor.matmul(ps1[:], lhsT=w1t[:, k, :], rhs=hTs[hf][k][:],
                                 start=(k == 0), stop=(k == DC - 1))
            nc.scalar.activation(dst_ap, ps1[:], act_fn)
            return w1t

        for _rep in range(n_reps):
            # ---------------- Phase A: block attention -> h, hT ----------------
            if not phase_a:
                zp = ctx.enter_context(tc.tile_pool(name="zp", bufs=1))
                zt = zp.tile([P, TH], f32)
                nc.vector.memset(zt, 0.001)
                for hf in range(2):
                    for k in range(DC):
                        nc.scalar.activation(hTs[hf][k][:], zt[:], AF.Copy)
            if phase_a:
              with ExitStack() as ctxA:
                vpool = ctxA.enter_context(tc.tile_pool(name="vpool", bufs=9))
                spool = ctxA.enter_context(tc.tile_pool(name="spool", bufs=1))
                sqps = ctxA.enter_context(tc.tile_pool(name="sqps", bufs=1, space="PSUM"))
                small = ctxA.enter_context(tc.tile_pool(name="small", bufs=3))
                hpool = ctxA.enter_context(tc.tile_pool(name="hpool", bufs=2))
                psumT = ctxA.enter_context(tc.tile_pool(name="psumT", bufs=2, space="PSUM"))
                consts = ctxA.enter_context(tc.tile_pool(name="consts", bufs=1))

                ident = consts.tile([P, P], f32)
                make_identity(nc, ident)
                eps_t = consts.tile([P, 1], f32)
                nc.vector.memset(eps_t, EPS)
                nb_t = spool.tile([P, D], f32, name="dsc")
                nsw_b = bass.AP(tensor=nsw.tensor, offset=nsw.offset,
                                ap=[[0, P], *nsw.ap])
                nc.gpsimd.dma_start(out=nb_t, in_=nsw_b)
                pj_t = hpool.tile([P, D], f32, name="ht")
                pjw_b = bass.AP(tensor=pjw.tensor, offset=pjw.offset,
                                ap=[[0, P], *pjw.ap])
                nc.gpsimd.dma_start(out=pj_t, in_=pjw_b)
                nc.vector.tensor_mul(pw_b[:], nb_t[:], pj_t[:])

                def attn_tile(tt):
                    ss9 = small.tile([P, N1], f32, name="ss9")
                    dp9 = small.tile([P, N1], f32, name="dp9")
                    vts = []
                    for n in range(N1):
                        v = vpool.tile([P, D], f32, name="vt")
                        nc.sync.dma_start(out=v, in_=vb[n, tt * P:(tt + 1) * P, :])
                        vts.append(v)
                        sq = sqps.tile([P, D], f32, name="sq")
                        nc.scalar.activation(sq[:], v[:], AF.Square,
                                             accum_out=ss9[:, n:n + 1])
                        dsc = spool.tile([P, D], f32, name="dsc")
                        nc.vector.scalar_tensor_tensor(
                            out=dsc[:], in0=v[:], scalar=1.0, in1=pw_b[:],
                            op0=ALU.mult, op1=ALU.mult, accum_out=dp9[:, n:n + 1])
                    rms9 = small.tile([P, N1], f32, name="rms9")
                    nc.scalar.activation(rms9[:], ss9[:], AF.Sqrt,
                                         bias=eps_t[:], scale=1.0 / D)
                    inv9 = small.tile([P, N1], f32, name="inv9")
                    nc.vector.reciprocal(inv9[:], rms9[:])
                    lg9 = small.tile([P, N1], f32, name="lg9")
                    nc.vector.tensor_mul(lg9[:], dp9[:], inv9[:])
                    mx1 = small.tile([P, 1], f32, name="mx1")
                    nc.vector.tensor_reduce(mx1[:], lg9[:], axis=mybir.AxisListType.X,
                                            op=ALU.max)
                    nc.vector.tensor_scalar_sub(lg9[:], lg9[:], mx1[:])
                    e9 = small.tile([P, N1], f32, name="e9")
                    se1 = small.tile([P, 1], f32, name="se1")
                    nc.scalar.activation(e9[:], lg9[:], AF.Exp, accum_out=se1[:])
                    invs = small.tile([P, 1], f32, name="invs")
                    nc.vector.reciprocal(invs[:], se1[:])
                    al9 = small.tile([P, N1], f32, name="al9")
                    nc.vector.tensor_scalar_mul(al9[:], e9[:], invs[:])

                    h_t = hpool.tile([P, D], f32, name="ht")
                    nc.vector.tensor_scalar_mul(h_t[:], vts[0][:], al9[:, 0:1])
                    for n in range(1, N1):
                        nc.vector.scalar_tensor_tensor(
                            out=h_t[:], in0=vts[n][:], scalar=al9[:, n:n + 1],
                            in1=h_t[:], op0=ALU.mult, op1=ALU.add)
                    nc.sync.dma_start(out=h_out_t[tt], in_=h_t[:])
                    hf, col = divmod(tt, 2)
                    for k in range(DC):
                        pst = psumT.tile([P, P], f32, name="pst")
                        nc.tensor.transpose(pst[:], h_t[:, k * P:(k + 1) * P], ident[:])
                        nc.scalar.activation(
                            hTs[hf][k][:, col * P:(col + 1) * P], pst[:], AF.Copy)

                attn_tile(0)
                attn_tile(1)
                if phase_b and overlap:
                    # MM1 over token-half A for every f-chunk, spilled to DRAM,
                    # overlapping attention tiles 2,3 on DVE/ACT.
                    for fc in range(FC):
                        a_sb = aspp.tile([P, TH], f32r, name="asp")
                        mm1_half(0, fc, a_sb[:])
                        nc.scalar.dma_start(out=act_d[fc], in_=a_sb[:])
                attn_tile(2)
                attn_tile(3)

            # ---------------- Phase B: FFN (f32r) + residual ----------------
            if phase_b:
              with ExitStack() as ctxB:
                w2p = ctxB.enter_context(tc.tile_pool(name="w2p", bufs=4))
                actap = ctxB.enter_context(tc.tile_pool(name="actap", bufs=FG + 4))
                actbp = ctxB.enter_context(tc.tile_pool(name="actbp", bufs=FG + 4))
                oap = ctxB.enter_context(tc.tile_pool(name="oap", bufs=1))
                evp = ctxB.enter_context(tc.tile_pool(name="evp", bufs=4))
                ptp = ctxB.enter_context(tc.tile_pool(name="ptp", bufs=4))
                ps2p = ctxB.enter_context(tc.tile_pool(name="ps2p", bufs=4, space="PSUM"))

                out_acc = [oap.tile([P, D], f32, name=f"oa{m}") for m in range(TT)]

                for g in range(NG):
                    act_a, act_b = [], []
                    for fcl in range(FG):
                        gfc = g * FG + fcl
                        if overlap:
                            # readback half-A act, compute half-B act
                            aa = actap.tile([P, TH], f32r, name="acta")
                            nc.sync.dma_start(out=aa, in_=act_d[gfc])
                            ab = actbp.tile([P, TH], f32r, name="actb")
                            mm1_half(1, gfc, ab[:])
                        else:
                            aa = actap.tile([P, TH], f32r, name="acta")
                            ab = actbp.tile([P, TH], f32r, name="actb")
                            w1t = mm1_half(0, gfc, aa[:])
                            mm1_half(1, gfc, ab[:], w1t=w1t)
                        act_a.append(aa)
                        act_b.append(ab)

                    for q in range(NQ):
                        ps2 = [ps2p.tile([P, 512], f32, name="ps2") for _ in range(TT)]
                        for fcl in range(FG):
                            gfc = g * FG + fcl
                            w2t = w2p.tile([P, 512], f32r, name="w2t")
                            nc.sync.dma_start(
                                out=w2t,
                                in_=w2[gfc * P:(gfc + 1) * P, q * 512:(q + 1) * 512])
                            for m in range(TT):
                                src = act_a[fcl] if m < 2 else act_b[fcl]
                                nc.tensor.matmul(
                                    ps2[m][:],
                                    lhsT=src[:, (m % 2) * P:(m % 2 + 1) * P],
                                    rhs=w2t[:],
                                    start=(fcl == 0), stop=(fcl == FG - 1))
                        for m in range(TT):
                            if g == 0:
                                nc.vector.tensor_copy(
                                    out_acc[m][:, q * 512:(q + 1) * 512], ps2[m][:])
                            elif g < NG - 1:
                                nc.vector.tensor_add(
                                    out_acc[m][:, q * 512:(q + 1) * 512], ps2[m][:],
                                    out_acc[m][:, q * 512:(q + 1) * 512])
                            else:
                                ev = evp.tile([P, 512], f32, name="ev")
                                nc.vector.tensor_add(
                                    ev[:], ps2[m][:],
                                    out_acc[m][:, q * 512:(q + 1) * 512])
                                pt = ptp.tile([P, 512], f32, name="pt")
                                nc.sync.dma_start(
                                    out=pt,
                                    in_=vb[NB, m * P:(m + 1) * P, q * 512:(q + 1) * 512])
                                nc.vector.tensor_add(ev[:], ev[:], pt[:])
                                nc.sync.dma_start(
                                    out=np_out[m * P:(m + 1) * P, q * 512:(q + 1) * 512],
                                    in_=ev[:])

    nc.compile()
    return nc


_NC = None


def _get_nc():
    global _NC
    if _NC is None:
        _NC = build_nc()
    return _NC


def kernel(blocks, partial_block, proj_w, norm_scale, ffn_w1, ffn_w2):
    blocks = np.ascontiguousarray(np.asarray(blocks, dtype=np.float32)).reshape(NB, TOK, D)
    pb = np.ascontiguousarray(np.asarray(partial_block, dtype=np.float32)).reshape(TOK, D)
    w1r = retile_w1(round_f32r(np.asarray(ffn_w1, dtype=np.float32)))
    w2r = round_f32r(np.asarray(ffn_w2, dtype=np.float32))
    pjw = np.ascontiguousarray(np.asarray(proj_w, dtype=np.float32))
    nsw = np.ascontiguousarray(np.asarray(norm_scale, dtype=np.float32))

    in_maps = []
    for c in range(N_CORES):
        sl = slice(c * TPC, (c + 1) * TPC)
        vbc = np.concatenate([blocks[:, sl], pb[None, sl]], axis=0)
        in_maps.append({"vb": vbc, "w1": w1r, "w2": w2r, "pjw": pjw, "nsw": nsw})

    nc = _get_nc()
    res = run_bass_kernel_spmd(nc, in_maps, list(range(N_CORES)))
    h = np.concatenate([r["h_out"] for r in res.results], axis=0).reshape(B, T, D)
    npar = np.concatenate([r["np_out"] for r in res.results], axis=0).reshape(B, T, D)
    return h, npar



# revision 2
# speedup vs baseline: 2.9706x; 2.9706x over previous
"""Trainium2 Bass kernel for BlockAttnResLayer — all-bf16 version.

Computation (reference):
  V = concat([blocks, partial[None]])            # [9, B*T, D]
  rms = sqrt(mean(V^2, -1) + 1e-8)
  logits[n,t] = (V[n,t,:] . (norm_scale*proj_w)) / rms[n,t]
  alpha = softmax(logits, axis=n)
  h = sum_n alpha * V
  f = gelu(h @ W1) @ W2                          # tanh-approx gelu
  new_partial = partial + f
  returns (h, new_partial)

Sharding: pure data-parallel over tokens (B*T = 4096 -> 512/core on 8 cores).
Weights replicated, host-converted to bf16 (halves weight DMA vs fp32/f32r;
PE rate for bf16 == f32r == 1 row/cycle).  V host-converted to bf16 (halves
the attention stream + 2x DVE rate on 16-bit ops).

Structure per core (512 tokens = 4 tiles of 128):
  - attention per token tile: 9 V tiles -> squares (ACT, accum), dot with
    norm_scale*proj_w (DVE, accum), softmax over 9 blocks, weighted sum
    (DVE bf16), h store; PE transposes h 128x128 chunks into hT [d, tok].
  - MM1: ps1[f128, tok256] = sum_k W1T-tile @ hT-tile (bf16), gelu -> actT
    in SBUF (bf16, [128, 64, 512] = 4 MB, no DRAM spill).  The first K_OVL
    f-chunks of token-half A are emitted between attention tiles 1 and 2 so
    the PE overlaps the tail of attention; their W1 tiles are re-streamed
    for half B (+K_OVL*0.5 MB DMA).
  - MM2: 2 passes over d-halves; each pass accumulates all 64 f-chunks into
    8 PSUM banks (2 d-quarters x 4 token tiles), W2 streamed once total;
    evac fuses +partial (DVE STT) and stores np_out.
"""
import numpy as np
from contextlib import ExitStack

import ml_dtypes

import concourse.bass as bass
import concourse.bacc as bacc
import concourse.tile as tile
from concourse import mybir
from concourse.bass_utils import run_bass_kernel_spmd
from concourse.masks import make_identity

f32 = mybir.dt.float32
bf16 = mybir.dt.bfloat16
AF = mybir.ActivationFunctionType
ALU = mybir.AluOpType
BF = ml_dtypes.bfloat16

N_CORES = 8
NB = 8            # completed blocks
N1 = 9            # blocks + partial
B, T, D, F = 2, 2048, 2048, 8192
TOK = B * T       # 4096
TPC = TOK // N_CORES  # 512 tokens per core
P = 128
TT = TPC // P     # 4 token tiles per core
TH = TPC // 2     # 256-token halves
DC = D // P       # 16 d-chunks
FC = F // P       # 64 f-chunks
K_OVL = 16        # f-chunks of half-A MM1 overlapped with attention tail
EPS = 1e-8


def build_nc(n_reps: int = 1, k_ovl: int = K_OVL):
    nc = bacc.Bacc("TRN2", target_bir_lowering=False, debug=False,
                   num_devices=N_CORES)
    vb = nc.dram_tensor("vb", [N1, TPC, D], bf16, kind="ExternalInput").ap()
    # w1[fc, p, kc, m] = W1[kc*128+p, fc*128+m]
    w1 = nc.dram_tensor("w1", [FC, P, DC, P], bf16, kind="ExternalInput").ap()
    # w2[qh, fc, p, dq] = W2[fc*128+p, qh*1024+dq]
    w2 = nc.dram_tensor("w2", [2, FC, P, 1024], bf16, kind="ExternalInput").ap()
    # pw = norm_scale * proj_w (host-fused)
    pw = nc.dram_tensor("pw", [D], bf16, kind="ExternalInput").ap()
    h_out = nc.dram_tensor("h_out", [TPC, D], bf16, kind="ExternalOutput").ap()
    np_out = nc.dram_tensor("np_out", [TPC, D], f32, kind="ExternalOutput").ap()

    with tile.TileContext(nc) as tc, ExitStack() as ctx:
        outer = ctx.enter_context(tc.tile_pool(name="outer", bufs=1))
        consts = ctx.enter_context(tc.tile_pool(name="consts", bufs=1))
        ident = consts.tile([P, P], bf16)
        make_identity(nc, ident)
        eps_t = consts.tile([P, 1], f32)
        nc.vector.memset(eps_t, EPS)
        pw_b = consts.tile([P, D], bf16)
        pw_bcast = bass.AP(tensor=pw.tensor, offset=pw.offset,
                           ap=[[0, P], *pw.ap])
        nc.gpsimd.dma_start(out=pw_b, in_=pw_bcast)

        # transposed h: [d-part, (kc), tok] and gelu activations [f-part, (fc), tok]
        hT = outer.tile([P, DC, TPC], bf16, name="hT")
        actT = outer.tile([P, FC, TPC], bf16, name="actT")
        pk = [outer.tile([P, D], bf16, name=f"pk{m}") for m in range(TT)]

        for _rep in range(n_reps):
          with ExitStack() as ctxM:
            w1p = ctxM.enter_context(tc.tile_pool(name="w1p", bufs=3))
            ps1p = ctxM.enter_context(tc.tile_pool(name="ps1p", bufs=2,
                                                   space="PSUM"))

            def mm1_fc(fc, hf, w1t=None):
                if w1t is None:
                    w1t = w1p.tile([P, DC, P], bf16, name="w1t")
                    nc.scalar.dma_start(out=w1t, in_=w1[fc])
                ps1 = ps1p.tile([P, TH], f32, name="ps1")
                for k in range(DC):
                    nc.tensor.matmul(ps1[:], lhsT=w1t[:, k, :],
                                     rhs=hT[:, k, hf * TH:(hf + 1) * TH],
                                     start=(k == 0), stop=(k == DC - 1))
                nc.scalar.activation(actT[:, fc, hf * TH:(hf + 1) * TH],
                                     ps1[:], AF.Gelu_apprx_tanh)
                return w1t

            with ExitStack() as ctxA:
                vpool = ctxA.enter_context(tc.tile_pool(name="vpool", bufs=12))
                sqp = ctxA.enter_context(tc.tile_pool(name="sqp", bufs=2))
                dscp = ctxA.enter_context(tc.tile_pool(name="dscp", bufs=2))
                small = ctxA.enter_context(tc.tile_pool(name="small", bufs=24))
                hp = ctxA.enter_context(tc.tile_pool(name="hp", bufs=2))
                psT = ctxA.enter_context(tc.tile_pool(name="psT", bufs=2,
                                                      space="PSUM"))

                def attn_tile(tt):
                    sl = slice(tt * P, (tt + 1) * P)
                    ss9 = small.tile([P, N1], f32, name="ss9")
                    dp9 = small.tile([P, N1], f32, name="dp9")
                    vts = []
                    for n in range(N1):
                        v = pk[tt] if n == NB else vpool.tile([P, D], bf16,
                                                              name="vt")
                        nc.sync.dma_start(out=v, in_=vb[n, sl, :])
                        vts.append(v)
                        sq = sqp.tile([P, D], bf16, name="sq")
                        nc.scalar.activation(sq[:], v[:], AF.Square,
                                             accum_out=ss9[:, n:n + 1])
                        dsc = dscp.tile([P, D], bf16, name="dsc")
                        nc.vector.scalar_tensor_tensor(
                            out=dsc[:], in0=v[:], scalar=1.0, in1=pw_b[:],
                            op0=ALU.mult, op1=ALU.mult,
                            accum_out=dp9[:, n:n + 1])
                    rms9 = small.tile([P, N1], f32, name="rms9")
                    nc.scalar.activation(rms9[:], ss9[:], AF.Sqrt,
                                         bias=eps_t[:], scale=1.0 / D)
                    inv9 = small.tile([P, N1], f32, name="inv9")
                    nc.vector.reciprocal(inv9[:], rms9[:])
                    lg9 = small.tile([P, N1], f32, name="lg9")
                    nc.vector.tensor_mul(lg9[:], dp9[:], inv9[:])
                    mx1 = small.tile([P, 1], f32, name="mx1")
                    nc.vector.tensor_reduce(mx1[:], lg9[:],
                                            axis=mybir.AxisListType.X,
                                            op=ALU.max)
                    nc.vector.tensor_scalar_sub(lg9[:], lg9[:], mx1[:])
                    e9 = small.tile([P, N1], f32, name="e9")
                    se1 = small.tile([P, 1], f32, name="se1")
                    nc.scalar.activation(e9[:], lg9[:], AF.Exp,
                                         accum_out=se1[:])
                    invs = small.tile([P, 1], f32, name="invs")
                    nc.vector.reciprocal(invs[:], se1[:])
                    al9 = small.tile([P, N1], f32, name="al9")
                    nc.vector.tensor_scalar_mul(al9[:], e9[:], invs[:])

                    h_t = hp.tile([P, D], bf16, name="ht")
                    nc.vector.tensor_scalar_mul(h_t[:], vts[0][:],
                                                al9[:, 0:1])
                    for n in range(1, N1):
                        nc.vector.scalar_tensor_tensor(
                            out=h_t[:], in0=vts[n][:], scalar=al9[:, n:n + 1],
                            in1=h_t[:], op0=ALU.mult, op1=ALU.add)
                    nc.scalar.dma_start(out=h_out[sl, :], in_=h_t[:])
                    for k in range(DC):
                        pst = psT.tile([P, P], bf16, name="pst")
                        nc.tensor.transpose(pst[:], h_t[:, k * P:(k + 1) * P],
                                            ident[:])
                        nc.scalar.activation(hT[:, k, tt * P:(tt + 1) * P],
                                             pst[:], AF.Copy)

                attn_tile(0)
                attn_tile(1)
                # overlap: half-A MM1 for the first k_ovl f-chunks runs on
                # the PE while attention tiles 2,3 occupy DVE/ACT/DMA.
                for fc in range(k_ovl):
                    mm1_fc(fc, 0)
                attn_tile(2)
                attn_tile(3)

            for fc in range(k_ovl, FC):
                w1t = mm1_fc(fc, 0)
                mm1_fc(fc, 1, w1t=w1t)
            for fc in range(k_ovl):
                mm1_fc(fc, 1)

          # ---------------- MM2 + residual ----------------
          with ExitStack() as ctxB:
            w2p = ctxB.enter_context(tc.tile_pool(name="w2p", bufs=6))
            ps2p = ctxB.enter_context(tc.tile_pool(name="ps2p", bufs=8,
                                                   space="PSUM"))
            evp = ctxB.enter_context(tc.tile_pool(name="evp", bufs=4))

            for qh in range(2):
                ps2 = [ps2p.tile([P, 512], f32, name="ps2")
                       for _ in range(8)]
                for fc in range(FC):
                    w2t = w2p.tile([P, 1024], bf16, name="w2t")
                    nc.scalar.dma_start(out=w2t, in_=w2[qh, fc])
                    for q2 in range(2):
                        for m in range(TT):
                            nc.tensor.matmul(
                                ps2[q2 * TT + m][:],
                                lhsT=actT[:, fc, m * P:(m + 1) * P],
                                rhs=w2t[:, q2 * 512:(q2 + 1) * 512],
                                start=(fc == 0), stop=(fc == FC - 1))
                for q2 in range(2):
                    for m in range(TT):
                        col = qh * 1024 + q2 * 512
                        ev = evp.tile([P, 512], f32, name="ev")
                        nc.vector.scalar_tensor_tensor(
                            out=ev[:], in0=ps2[q2 * TT + m][:], scalar=1.0,
                            in1=pk[m][:, col:col + 512],
                            op0=ALU.mult, op1=ALU.add)
                        nc.gpsimd.dma_start(
                            out=np_out[m * P:(m + 1) * P, col:col + 512],
                            in_=ev[:])

    nc.compile()
    return nc


def prep_in_maps(inputs: dict) -> list[dict]:
    blocks = np.asarray(inputs["blocks"], np.float32).reshape(NB, TOK, D)
    pb = np.asarray(inputs["partial_block"], np.float32).reshape(TOK, D)
    w1 = np.asarray(inputs["ffn_w1"], np.float32)
    w2 = np.asarray(inputs["ffn_w2"], np.float32)
    w1h = np.ascontiguousarray(
        w1.reshape(DC, P, FC, P).transpose(2, 1, 0, 3)).astype(BF)
    w2h = np.ascontiguousarray(
        w2.reshape(FC, P, 2, 1024).transpose(2, 0, 1, 3)).astype(BF)
    pwh = (np.asarray(inputs["proj_w"], np.float32)
           * np.asarray(inputs["norm_scale"], np.float32)).astype(BF)
    in_maps = []
    for c in range(N_CORES):
        sl = slice(c * TPC, (c + 1) * TPC)
        vbc = np.concatenate([blocks[:, sl], pb[None, sl]],
                             axis=0).astype(BF)
        in_maps.append({"vb": vbc, "w1": w1h, "w2": w2h, "pw": pwh})
    return in_maps


_NC = None


def _get_nc():
    global _NC
    if _NC is None:
        _NC = build_nc()
    return _NC


def kernel(blocks, partial_block, proj_w, norm_scale, ffn_w1, ffn_w2):
    in_maps = prep_in_maps(dict(blocks=blocks, partial_block=partial_block,
                                proj_w=proj_w, norm_scale=norm_scale,
                                ffn_w1=ffn_w1, ffn_w2=ffn_w2))
    nc = _get_nc()
    res = run_bass_kernel_spmd(nc, in_maps, list(range(N_CORES)))
    h = np.concatenate([np.asarray(r["h_out"], dtype=np.float32)
                        for r in res.results], axis=0).reshape(B, T, D)
    npar = np.concatenate([r["np_out"] for r in res.results],
                          axis=0).reshape(B, T, D)
    return h, npar


# revision 7
# speedup vs baseline: 3.6509x; 1.2290x over previous
"""Trainium2 Bass kernel for BlockAttnResLayer — all-f16, steady-state pipelined.

See kernel.py docstring for the computation.  Differences vs v1:
  - MM1 full-width (N=512) — W1 streamed once, PE near roofline.
  - MM2 as 4 quarter-passes (d-quarters) x 4 PSUM banks, W2 streamed once.
  - All pools persistent so consecutive reps pipeline: rep k's attention
    (DVE/ACT/DMA) overlaps rep k-1's MM2 (PE).
  - h accumulated and stored in fp32 (error ~3.5e-3 vs 1.06e-2 for f16).
PSUM budget: ps1 2 banks + ps2 4 banks + transpose 2 banks(packed) <= 8.
"""
import numpy as np
from contextlib import ExitStack

import ml_dtypes

import concourse.bass as bass
import concourse.bacc as bacc
import concourse.tile as tile
from concourse import mybir
from concourse.bass_utils import run_bass_kernel_spmd
from concourse.masks import make_identity

f32 = mybir.dt.float32
f16 = mybir.dt.float16
AF = mybir.ActivationFunctionType
ALU = mybir.AluOpType
F16 = np.float16

N_CORES = 8
NB = 8            # completed blocks
N1 = 9            # blocks + partial
B, T, D, F = 2, 2048, 2048, 8192
TOK = B * T       # 4096
TPC = TOK // N_CORES  # 512 tokens per core
P = 128
TT = TPC // P     # 4 token tiles per core
DC = D // P       # 16 d-chunks
FC = F // P       # 64 f-chunks
NQ = D // 512     # 4 output column quarters
EPS = 1e-8


def build_nc(n_reps: int = 1, do_attn: bool = True, do_mm1: bool = True,
             do_mm2: bool = True):
    nc = bacc.Bacc("TRN2", target_bir_lowering=False, debug=False,
                   num_devices=N_CORES)
    vb = nc.dram_tensor("vb", [N1, TPC, D], f16, kind="ExternalInput").ap()
    # w1[fc, p, kc, m] = W1[kc*128+p, fc*128+m]
    w1 = nc.dram_tensor("w1", [FC, P, DC, P], f16, kind="ExternalInput").ap()
    # w2[q, fc, p, dq] = W2[fc*128+p, q*512+dq]
    w2 = nc.dram_tensor("w2", [NQ, FC, P, 512], f16, kind="ExternalInput").ap()
    # pw = norm_scale * proj_w (host-fused)
    pw = nc.dram_tensor("pw", [D], f16, kind="ExternalInput").ap()
    h_out = nc.dram_tensor("h_out", [TPC, D], f16, kind="ExternalOutput").ap()
    np_out = nc.dram_tensor("np_out", [TPC, D], f16, kind="ExternalOutput").ap()

    with tile.TileContext(nc) as tc, ExitStack() as ctx:
        outer = ctx.enter_context(tc.tile_pool(name="outer", bufs=1))
        ident = outer.tile([P, P], f16)
        make_identity(nc, ident)
        eps_t = outer.tile([P, 1], f32)
        nc.vector.memset(eps_t, EPS)
        pw_b = outer.tile([P, D], f16)
        pw_bcast = bass.AP(tensor=pw.tensor, offset=pw.offset,
                           ap=[[0, P], *pw.ap])
        nc.gpsimd.dma_start(out=pw_b, in_=pw_bcast)

        hT = outer.tile([P, DC, TPC], f16, name="hT")
        actT = outer.tile([P, FC, TPC], f16, name="actT")
        pk = [outer.tile([P, D], f16, name=f"pk{m}") for m in range(TT)]

        w1p = ctx.enter_context(tc.tile_pool(name="w1p", bufs=3))
        ps1p = ctx.enter_context(tc.tile_pool(name="ps1p", bufs=2, space="PSUM"))
        vpool = ctx.enter_context(tc.tile_pool(name="vpool", bufs=9))
        sqp = ctx.enter_context(tc.tile_pool(name="sqp", bufs=2))
        dscp = ctx.enter_context(tc.tile_pool(name="dscp", bufs=2))
        small = ctx.enter_context(tc.tile_pool(name="small", bufs=24))
        hp = ctx.enter_context(tc.tile_pool(name="hp", bufs=2))
        psT = ctx.enter_context(tc.tile_pool(name="psT", bufs=2, space="PSUM"))
        w2p = ctx.enter_context(tc.tile_pool(name="w2p", bufs=6))
        ps2p = ctx.enter_context(tc.tile_pool(name="ps2p", bufs=4, space="PSUM"))
        evp = ctx.enter_context(tc.tile_pool(name="evp", bufs=4))

        def attn_tile(tt):
            sl = slice(tt * P, (tt + 1) * P)
            ss9 = small.tile([P, N1], f32, name="ss9")
            dp9 = small.tile([P, N1], f32, name="dp9")
            vts = []
            for n in range(N1):
                v = pk[tt] if n == NB else vpool.tile([P, D], f16, name="vt")
                nc.sync.dma_start(out=v, in_=vb[n, sl, :])
                vts.append(v)
                sq = sqp.tile([P, D], f16, name="sq")
                nc.scalar.activation(sq[:], v[:], AF.Square,
                                     accum_out=ss9[:, n:n + 1])
                dsc = dscp.tile([P, D], f16, name="dsc")
                nc.vector.scalar_tensor_tensor(
                    out=dsc[:], in0=v[:], scalar=1.0, in1=pw_b[:],
                    op0=ALU.mult, op1=ALU.mult, accum_out=dp9[:, n:n + 1])
            rms9 = small.tile([P, N1], f32, name="rms9")
            nc.scalar.activation(rms9[:], ss9[:], AF.Sqrt,
                                 bias=eps_t[:], scale=1.0 / D)
            inv9 = small.tile([P, N1], f32, name="inv9")
            nc.vector.reciprocal(inv9[:], rms9[:])
            lg9 = small.tile([P, N1], f32, name="lg9")
            nc.vector.tensor_mul(lg9[:], dp9[:], inv9[:])
            mx1 = small.tile([P, 1], f32, name="mx1")
            nc.vector.tensor_reduce(mx1[:], lg9[:], axis=mybir.AxisListType.X,
                                    op=ALU.max)
            nc.vector.tensor_scalar_sub(lg9[:], lg9[:], mx1[:])
            e9 = small.tile([P, N1], f32, name="e9")
            se1 = small.tile([P, 1], f32, name="se1")
            nc.scalar.activation(e9[:], lg9[:], AF.Exp, accum_out=se1[:])
            invs = small.tile([P, 1], f32, name="invs")
            nc.vector.reciprocal(invs[:], se1[:])
            al9 = small.tile([P, N1], f32, name="al9")
            nc.vector.tensor_scalar_mul(al9[:], e9[:], invs[:])

            h_t = hp.tile([P, D], f16, name="ht")
            nc.vector.tensor_scalar_mul(h_t[:], vts[0][:], al9[:, 0:1])
            for n in range(1, N1):
                nc.vector.scalar_tensor_tensor(
                    out=h_t[:], in0=vts[n][:], scalar=al9[:, n:n + 1],
                    in1=h_t[:], op0=ALU.mult, op1=ALU.add)
            nc.scalar.dma_start(out=h_out[sl, :], in_=h_t[:])
            for k in range(DC):
                pst = psT.tile([P, P], f16, name="pst")
                nc.tensor.transpose(pst[:], h_t[:, k * P:(k + 1) * P],
                                    ident[:])
                nc.scalar.activation(hT[:, k, tt * P:(tt + 1) * P],
                                     pst[:], AF.Copy)

        for _rep in range(n_reps):
            # ---------------- attention ----------------
            if do_attn:
                for tt in range(TT):
                    attn_tile(tt)
            # ---------------- MM1 + gelu ----------------
            for fc in range(FC if do_mm1 else 0):
                w1t = w1p.tile([P, DC, P], f16, name="w1t")
                nc.scalar.dma_start(out=w1t, in_=w1[fc])
                ps1 = ps1p.tile([P, TPC], f32, name="ps1")
                for k in range(DC):
                    nc.tensor.matmul(ps1[:], lhsT=w1t[:, k, :],
                                     rhs=hT[:, k, :],
                                     start=(k == 0), stop=(k == DC - 1))
                nc.scalar.activation(actT[:, fc, :], ps1[:],
                                     AF.Gelu_apprx_tanh)
            # ---------------- MM2 + residual ----------------
            for q in range(NQ if do_mm2 else 0):
                ps2 = [ps2p.tile([P, 512], f32, name="ps2")
                       for _ in range(TT)]
                for fc in range(FC):
                    w2t = w2p.tile([P, 512], f16, name="w2t")
                    nc.scalar.dma_start(out=w2t, in_=w2[q, fc])
                    for m in range(TT):
                        nc.tensor.matmul(
                            ps2[m][:],
                            lhsT=actT[:, fc, m * P:(m + 1) * P],
                            rhs=w2t[:],
                            start=(fc == 0), stop=(fc == FC - 1))
                col = q * 512
                for m in range(TT):
                    ev = evp.tile([P, 512], f16, name="ev")
                    nc.vector.scalar_tensor_tensor(
                        out=ev[:], in0=ps2[m][:], scalar=1.0,
                        in1=pk[m][:, col:col + 512],
                        op0=ALU.mult, op1=ALU.add)
                    nc.gpsimd.dma_start(
                        out=np_out[m * P:(m + 1) * P, col:col + 512],
                        in_=ev[:])

    nc.compile()
    return nc


def prep_in_maps(inputs: dict) -> list[dict]:
    blocks = np.asarray(inputs["blocks"], np.float32).reshape(NB, TOK, D)
    pb = np.asarray(inputs["partial_block"], np.float32).reshape(TOK, D)
    w1 = np.asarray(inputs["ffn_w1"], np.float32)
    w2 = np.asarray(inputs["ffn_w2"], np.float32)
    w1h = np.ascontiguousarray(
        w1.reshape(DC, P, FC, P).transpose(2, 1, 0, 3)).astype(F16)
    w2h = np.ascontiguousarray(
        w2.reshape(FC, P, NQ, 512).transpose(2, 0, 1, 3)).astype(F16)
    pwh = (np.asarray(inputs["proj_w"], np.float32)
           * np.asarray(inputs["norm_scale"], np.float32)).astype(F16)
    in_maps = []
    for c in range(N_CORES):
        sl = slice(c * TPC, (c + 1) * TPC)
        vbc = np.concatenate([blocks[:, sl], pb[None, sl]],
                             axis=0).astype(F16)
        in_maps.append({"vb": vbc, "w1": w1h, "w2": w2h, "pw": pwh})
    return in_maps


_NC = None


def _get_nc():
    global _NC
    if _NC is None:
        _NC = build_nc()
    return _NC


def kernel(blocks, partial_block, proj_w, norm_scale, ffn_w1, ffn_w2):
    in_maps = prep_in_maps(dict(blocks=blocks, partial_block=partial_block,
                                proj_w=proj_w, norm_scale=norm_scale,
                                ffn_w1=ffn_w1, ffn_w2=ffn_w2))
    nc = _get_nc()
    res = run_bass_kernel_spmd(nc, in_maps, list(range(N_CORES)))
    h = np.concatenate([np.asarray(r["h_out"], dtype=np.float32)
                        for r in res.results], axis=0).reshape(B, T, D)
    npar = np.concatenate([np.asarray(r["np_out"], dtype=np.float32)
                           for r in res.results], axis=0).reshape(B, T, D)
    return h, npar


# revision 12
# speedup vs baseline: 3.7077x; 1.0156x over previous
"""Trainium2 Bass kernel for BlockAttnResLayer — all-f16, steady-state pipelined.

See kernel.py docstring for the computation.  Differences vs v1:
  - MM1 full-width (N=512) — W1 streamed once, PE near roofline.
  - MM2 as 4 quarter-passes (d-quarters) x 4 PSUM banks, W2 streamed once.
  - All pools persistent so consecutive reps pipeline: rep k's attention
    (DVE/ACT/DMA) overlaps rep k-1's MM2 (PE).
  - h accumulated and stored in fp32 (error ~3.5e-3 vs 1.06e-2 for f16).
PSUM budget: ps1 2 banks + ps2 4 banks + transpose 2 banks(packed) <= 8.
"""
import numpy as np
from contextlib import ExitStack

import ml_dtypes

import concourse.bass as bass
import concourse.bacc as bacc
import concourse.tile as tile
from concourse import mybir
from concourse.bass_utils import run_bass_kernel_spmd
from concourse.masks import make_identity

f32 = mybir.dt.float32
f16 = mybir.dt.float16
AF = mybir.ActivationFunctionType
ALU = mybir.AluOpType
F16 = np.float16

N_CORES = 8
NB = 8            # completed blocks
N1 = 9            # blocks + partial
B, T, D, F = 2, 2048, 2048, 8192
TOK = B * T       # 4096
TPC = TOK // N_CORES  # 512 tokens per core
P = 128
TT = TPC // P     # 4 token tiles per core
DC = D // P       # 16 d-chunks
FC = F // P       # 64 f-chunks
NQ = D // 512     # 4 output column quarters
EPS = 1e-8


def build_nc(n_reps: int = 1, do_attn: bool = True, do_mm1: bool = True,
             do_mm2: bool = True):
    nc = bacc.Bacc("TRN2", target_bir_lowering=False, debug=False,
                   num_devices=N_CORES)
    vb = nc.dram_tensor("vb", [N1, TPC, D], f16, kind="ExternalInput").ap()
    # w1[fc, p, kc, m] = W1[kc*128+p, fc*128+m]
    w1 = nc.dram_tensor("w1", [FC, P, DC, P], f16, kind="ExternalInput").ap()
    # w2[q, fc, p, dq] = W2[fc*128+p, q*512+dq]
    w2 = nc.dram_tensor("w2", [NQ, FC, P, 512], f16, kind="ExternalInput").ap()
    # pw = norm_scale * proj_w (host-fused)
    pw = nc.dram_tensor("pw", [D], f16, kind="ExternalInput").ap()
    h_out = nc.dram_tensor("h_out", [TPC, D], f16, kind="ExternalOutput").ap()
    np_out = nc.dram_tensor("np_out", [TPC, D], f16, kind="ExternalOutput").ap()

    with tile.TileContext(nc) as tc, ExitStack() as ctx:
        outer = ctx.enter_context(tc.tile_pool(name="outer", bufs=1))
        ident = outer.tile([P, P], f16)
        make_identity(nc, ident)
        eps_t = outer.tile([P, 1], f32)
        nc.vector.memset(eps_t, EPS)
        pw_b = outer.tile([P, D], f16)
        pw_bcast = bass.AP(tensor=pw.tensor, offset=pw.offset,
                           ap=[[0, P], *pw.ap])
        nc.gpsimd.dma_start(out=pw_b, in_=pw_bcast)

        hT = outer.tile([P, DC, TPC], f16, name="hT")
        actT = outer.tile([P, FC, TPC], f16, name="actT")
        pk = [outer.tile([P, D], f16, name=f"pk{m}") for m in range(TT)]

        w1p = ctx.enter_context(tc.tile_pool(name="w1p", bufs=3))
        ps1p = ctx.enter_context(tc.tile_pool(name="ps1p", bufs=2, space="PSUM"))
        vpool = ctx.enter_context(tc.tile_pool(name="vpool", bufs=9))
        sqp = ctx.enter_context(tc.tile_pool(name="sqp", bufs=2))
        dscp = ctx.enter_context(tc.tile_pool(name="dscp", bufs=2))
        small = ctx.enter_context(tc.tile_pool(name="small", bufs=24))
        hp = ctx.enter_context(tc.tile_pool(name="hp", bufs=2))
        psT = ctx.enter_context(tc.tile_pool(name="psT", bufs=2, space="PSUM"))
        w2p = ctx.enter_context(tc.tile_pool(name="w2p", bufs=6))
        ps2p = ctx.enter_context(tc.tile_pool(name="ps2p", bufs=4, space="PSUM"))
        evp = ctx.enter_context(tc.tile_pool(name="evp", bufs=4))

        def attn_tile(tt):
            sl = slice(tt * P, (tt + 1) * P)
            ss9 = small.tile([P, N1], f32, name="ss9")
            dp9 = small.tile([P, N1], f32, name="dp9")
            vts = []
            for n in range(N1):
                v = pk[tt] if n == NB else vpool.tile([P, D], f16, name="vt")
                nc.sync.dma_start(out=v, in_=vb[n, sl, :])
                vts.append(v)
                sq = sqp.tile([P, D], f16, name="sq")
                nc.scalar.activation(sq[:], v[:], AF.Square,
                                     accum_out=ss9[:, n:n + 1])
                dsc = dscp.tile([P, D], f16, name="dsc")
                nc.vector.scalar_tensor_tensor(
                    out=dsc[:], in0=v[:], scalar=1.0, in1=pw_b[:],
                    op0=ALU.mult, op1=ALU.mult, accum_out=dp9[:, n:n + 1])
            rms9 = small.tile([P, N1], f32, name="rms9")
            nc.scalar.activation(rms9[:], ss9[:], AF.Sqrt,
                                 bias=eps_t[:], scale=1.0 / D)
            inv9 = small.tile([P, N1], f32, name="inv9")
            nc.vector.reciprocal(inv9[:], rms9[:])
            lg9 = small.tile([P, N1], f32, name="lg9")
            nc.vector.tensor_mul(lg9[:], dp9[:], inv9[:])
            mx1 = small.tile([P, 1], f32, name="mx1")
            nc.vector.tensor_reduce(mx1[:], lg9[:], axis=mybir.AxisListType.X,
                                    op=ALU.max)
            nc.vector.tensor_scalar_sub(lg9[:], lg9[:], mx1[:])
            e9 = small.tile([P, N1], f32, name="e9")
            se1 = small.tile([P, 1], f32, name="se1")
            nc.scalar.activation(e9[:], lg9[:], AF.Exp, accum_out=se1[:])
            invs = small.tile([P, 1], f32, name="invs")
            nc.vector.reciprocal(invs[:], se1[:])
            al9 = small.tile([P, N1], f32, name="al9")
            nc.vector.tensor_scalar_mul(al9[:], e9[:], invs[:])

            h_t = hp.tile([P, D], f16, name="ht")
            nc.vector.tensor_scalar_mul(h_t[:], vts[0][:], al9[:, 0:1])
            for n in range(1, N1):
                nc.vector.scalar_tensor_tensor(
                    out=h_t[:], in0=vts[n][:], scalar=al9[:, n:n + 1],
                    in1=h_t[:], op0=ALU.mult, op1=ALU.add)
            nc.scalar.dma_start(out=h_out[sl, :], in_=h_t[:])
            for k in range(DC):
                pst = psT.tile([P, P], f16, name="pst")
                nc.tensor.transpose(pst[:], h_t[:, k * P:(k + 1) * P],
                                    ident[:])
                nc.scalar.activation(hT[:, k, tt * P:(tt + 1) * P],
                                     pst[:], AF.Copy)

        for _rep in range(n_reps):
            # ---------------- attention ----------------
            if do_attn:
                for tt in range(TT):
                    attn_tile(tt)
            # ---------------- MM1 + gelu ----------------
            for fc in range(FC if do_mm1 else 0):
                w1t = w1p.tile([P, DC, P], f16, name="w1t")
                nc.scalar.dma_start(out=w1t, in_=w1[fc])
                ps1 = ps1p.tile([P, TPC], f32, name="ps1")
                for k in range(DC):
                    nc.tensor.matmul(ps1[:], lhsT=w1t[:, k, :],
                                     rhs=hT[:, k, :],
                                     start=(k == 0), stop=(k == DC - 1))
                nc.scalar.activation(actT[:, fc, :], ps1[:],
                                     AF.Gelu_apprx_tanh)
            # ---------------- MM2 + residual ----------------
            for q in range(NQ if do_mm2 else 0):
                ps2 = [ps2p.tile([P, 512], f32, name="ps2")
                       for _ in range(TT)]
                for fc in range(FC):
                    w2t = w2p.tile([P, 512], f16, name="w2t")
                    nc.scalar.dma_start(out=w2t, in_=w2[q, fc])
                    for m in range(TT):
                        nc.tensor.matmul(
                            ps2[m][:],
                            lhsT=actT[:, fc, m * P:(m + 1) * P],
                            rhs=w2t[:],
                            start=(fc == 0), stop=(fc == FC - 1))
                col = q * 512
                for m in range(TT):
                    ev = evp.tile([P, 512], f16, name="ev")
                    nc.vector.scalar_tensor_tensor(
                        out=ev[:], in0=ps2[m][:], scalar=1.0,
                        in1=pk[m][:, col:col + 512],
                        op0=ALU.mult, op1=ALU.add)
                    nc.gpsimd.dma_start(
                        out=np_out[m * P:(m + 1) * P, col:col + 512],
                        in_=ev[:])

    nc.compile()
    return nc


def prep_in_maps(inputs: dict) -> list[dict]:
    blocks = np.asarray(inputs["blocks"], np.float32).reshape(NB, TOK, D)
    pb = np.asarray(inputs["partial_block"], np.float32).reshape(TOK, D)
    w1 = np.asarray(inputs["ffn_w1"], np.float32)
    w2 = np.asarray(inputs["ffn_w2"], np.float32)
    w1h = np.ascontiguousarray(
        w1.reshape(DC, P, FC, P).transpose(2, 1, 0, 3)).astype(F16)
    w2h = np.ascontiguousarray(
        w2.reshape(FC, P, NQ, 512).transpose(2, 0, 1, 3)).astype(F16)
    pwh = (np.asarray(inputs["proj_w"], np.float32)
           * np.asarray(inputs["norm_scale"], np.float32)).astype(F16)
    in_maps = []
    for c in range(N_CORES):
        sl = slice(c * TPC, (c + 1) * TPC)
        vbc = np.concatenate([blocks[:, sl], pb[None, sl]],
                             axis=0).astype(F16)
        in_maps.append({"vb": vbc, "w1": w1h, "w2": w2h, "pw": pwh})
    return in_maps


_NC = None


def _get_nc():
    global _NC
    if _NC is None:
        _NC = build_nc()
    return _NC


def kernel(blocks, partial_block, proj_w, norm_scale, ffn_w1, ffn_w2):
    in_maps = prep_in_maps(dict(blocks=blocks, partial_block=partial_block,
                                proj_w=proj_w, norm_scale=norm_scale,
                                ffn_w1=ffn_w1, ffn_w2=ffn_w2))
    nc = _get_nc()
    res = run_bass_kernel_spmd(nc, in_maps, list(range(N_CORES)))
    h = np.concatenate([np.asarray(r["h_out"], dtype=np.float32)
                        for r in res.results], axis=0).reshape(B, T, D)
    npar = np.concatenate([np.asarray(r["np_out"], dtype=np.float32)
                           for r in res.results], axis=0).reshape(B, T, D)
    return h, npar


# revision 14
# speedup vs baseline: 4.3201x; 1.1652x over previous
"""Trainium2 Bass kernel for BlockAttnResLayer — all-f16, steady-state pipelined.

See kernel.py docstring for the computation.  Differences vs v1:
  - MM1 full-width (N=512) — W1 streamed once, PE near roofline.
  - MM2 as 4 quarter-passes (d-quarters) x 4 PSUM banks, W2 streamed once.
  - All pools persistent so consecutive reps pipeline: rep k's attention
    (DVE/ACT/DMA) overlaps rep k-1's MM2 (PE).
  - h accumulated and stored in fp32 (error ~3.5e-3 vs 1.06e-2 for f16).
PSUM budget: ps1 2 banks + ps2 4 banks + transpose 2 banks(packed) <= 8.
"""
import numpy as np
from contextlib import ExitStack

import ml_dtypes

import concourse.bass as bass
import concourse.bacc as bacc
import concourse.tile as tile
from concourse import mybir
from concourse.bass_utils import run_bass_kernel_spmd
from concourse.masks import make_identity

f32 = mybir.dt.float32
f16 = mybir.dt.float16
AF = mybir.ActivationFunctionType
ALU = mybir.AluOpType
F16 = np.float16

N_CORES = 8
NB = 8            # completed blocks
N1 = 9            # blocks + partial
B, T, D, F = 2, 2048, 2048, 8192
TOK = B * T       # 4096
TPC = TOK // N_CORES  # 512 tokens per core
P = 128
TT = TPC // P     # 4 token tiles per core
DC = D // P       # 16 d-chunks
FC = F // P       # 64 f-chunks
NQ = D // 512     # 4 output column quarters
EPS = 1e-8


def build_nc(n_reps: int = 1, do_attn: bool = True, do_mm1: bool = True,
             do_mm2: bool = True):
    nc = bacc.Bacc("TRN2", target_bir_lowering=False, debug=False,
                   num_devices=N_CORES)
    vb = nc.dram_tensor("vb", [N1, TPC, D], f16, kind="ExternalInput").ap()
    # w1[fc, p, kc, m] = W1[kc*128+p, fc*128+m]
    w1 = nc.dram_tensor("w1", [FC, P, DC, P], f16, kind="ExternalInput").ap()
    # w2[q, fc, p, dq] = W2[fc*128+p, q*512+dq]
    w2 = nc.dram_tensor("w2", [NQ, FC, P, 512], f16, kind="ExternalInput").ap()
    # pw = norm_scale * proj_w (host-fused)
    pw = nc.dram_tensor("pw", [D], f16, kind="ExternalInput").ap()
    h_out = nc.dram_tensor("h_out", [TPC, D], f16, kind="ExternalOutput").ap()
    np_out = nc.dram_tensor("np_out", [TPC, D], f16, kind="ExternalOutput").ap()

    with tile.TileContext(nc) as tc, ExitStack() as ctx:
        outer = ctx.enter_context(tc.tile_pool(name="outer", bufs=1))
        ident = outer.tile([P, P], f16)
        make_identity(nc, ident)
        eps_t = outer.tile([P, 1], f32)
        nc.vector.memset(eps_t, EPS)
        pw_b = outer.tile([P, D], f16)
        pw_bcast = bass.AP(tensor=pw.tensor, offset=pw.offset,
                           ap=[[0, P], *pw.ap])
        nc.gpsimd.dma_start(out=pw_b, in_=pw_bcast)

        hT = outer.tile([P, DC, TPC], f16, name="hT")
        actT = outer.tile([P, FC, TPC], f16, name="actT")
        pk = [outer.tile([P, D], f16, name=f"pk{m}") for m in range(TT)]

        w1p = ctx.enter_context(tc.tile_pool(name="w1p", bufs=3))
        ps1p = ctx.enter_context(tc.tile_pool(name="ps1p", bufs=2, space="PSUM"))
        vpool = ctx.enter_context(tc.tile_pool(name="vpool", bufs=9))
        sqp = ctx.enter_context(tc.tile_pool(name="sqp", bufs=2))
        dscp = ctx.enter_context(tc.tile_pool(name="dscp", bufs=2))
        small = ctx.enter_context(tc.tile_pool(name="small", bufs=24))
        hp = ctx.enter_context(tc.tile_pool(name="hp", bufs=2))
        psT = ctx.enter_context(tc.tile_pool(name="psT", bufs=2, space="PSUM"))
        w2p = ctx.enter_context(tc.tile_pool(name="w2p", bufs=6))
        ps2p = ctx.enter_context(tc.tile_pool(name="ps2p", bufs=4, space="PSUM"))
        evp = ctx.enter_context(tc.tile_pool(name="evp", bufs=4))

        def attn_tile(tt):
            sl = slice(tt * P, (tt + 1) * P)
            ss9 = small.tile([P, N1], f32, name="ss9")
            dp9 = small.tile([P, N1], f32, name="dp9")
            vts = []
            for n in range(N1):
                v = pk[tt] if n == NB else vpool.tile([P, D], f16, name="vt")
                nc.sync.dma_start(out=v, in_=vb[n, sl, :])
                vts.append(v)
                sq = sqp.tile([P, D], f16, name="sq")
                nc.scalar.activation(sq[:], v[:], AF.Square,
                                     accum_out=ss9[:, n:n + 1])
                dsc = dscp.tile([P, D], f16, name="dsc")
                nc.vector.scalar_tensor_tensor(
                    out=dsc[:], in0=v[:], scalar=1.0, in1=pw_b[:],
                    op0=ALU.mult, op1=ALU.mult, accum_out=dp9[:, n:n + 1])
            rms9 = small.tile([P, N1], f32, name="rms9")
            nc.scalar.activation(rms9[:], ss9[:], AF.Sqrt,
                                 bias=eps_t[:], scale=1.0 / D)
            inv9 = small.tile([P, N1], f32, name="inv9")
            nc.vector.reciprocal(inv9[:], rms9[:])
            lg9 = small.tile([P, N1], f32, name="lg9")
            nc.vector.tensor_mul(lg9[:], dp9[:], inv9[:])
            mx1 = small.tile([P, 1], f32, name="mx1")
            nc.vector.tensor_reduce(mx1[:], lg9[:], axis=mybir.AxisListType.X,
                                    op=ALU.max)
            nc.vector.tensor_scalar_sub(lg9[:], lg9[:], mx1[:])
            e9 = small.tile([P, N1], f32, name="e9")
            se1 = small.tile([P, 1], f32, name="se1")
            nc.scalar.activation(e9[:], lg9[:], AF.Exp, accum_out=se1[:])
            invs = small.tile([P, 1], f32, name="invs")
            nc.vector.reciprocal(invs[:], se1[:])
            al9 = small.tile([P, N1], f32, name="al9")
            nc.vector.tensor_scalar_mul(al9[:], e9[:], invs[:])

            h_t = hp.tile([P, D], f16, name="ht")
            nc.vector.tensor_scalar_mul(h_t[:], vts[0][:], al9[:, 0:1])
            for n in range(1, N1):
                nc.vector.scalar_tensor_tensor(
                    out=h_t[:], in0=vts[n][:], scalar=al9[:, n:n + 1],
                    in1=h_t[:], op0=ALU.mult, op1=ALU.add)
            nc.scalar.dma_start(out=h_out[sl, :], in_=h_t[:])
            for k in range(DC):
                pst = psT.tile([P, P], f16, name="pst")
                nc.tensor.transpose(pst[:], h_t[:, k * P:(k + 1) * P],
                                    ident[:])
                nc.scalar.activation(hT[:, k, tt * P:(tt + 1) * P],
                                     pst[:], AF.Copy)

        deferred = []

        def flush_deferred():
            for fn in deferred:
                fn()
            deferred.clear()

        for _rep in range(n_reps):
            # ---------------- attention ----------------
            if do_attn:
                for tt in range(TT):
                    attn_tile(tt)
            # rep k-1's last-quarter MM2 evacs run on DVE only after this
            # rep's attention DVE work, so attention overlaps MM2(k-1).
            flush_deferred()
            # ---------------- MM1 + gelu ----------------
            for fc in range(FC if do_mm1 else 0):
                w1t = w1p.tile([P, DC, P], f16, name="w1t")
                nc.scalar.dma_start(out=w1t, in_=w1[fc])
                ps1 = ps1p.tile([P, TPC], f32, name="ps1")
                for k in range(DC):
                    nc.tensor.matmul(ps1[:], lhsT=w1t[:, k, :],
                                     rhs=hT[:, k, :],
                                     start=(k == 0), stop=(k == DC - 1))
                nc.scalar.activation(actT[:, fc, :], ps1[:],
                                     AF.Gelu_apprx_tanh)
            # ---------------- MM2 + residual ----------------
            for q in range(NQ if do_mm2 else 0):
                ps2 = [ps2p.tile([P, 512], f32, name="ps2")
                       for _ in range(TT)]
                for fc in range(FC):
                    w2t = w2p.tile([P, 512], f16, name="w2t")
                    nc.scalar.dma_start(out=w2t, in_=w2[q, fc])
                    for m in range(TT):
                        nc.tensor.matmul(
                            ps2[m][:],
                            lhsT=actT[:, fc, m * P:(m + 1) * P],
                            rhs=w2t[:],
                            start=(fc == 0), stop=(fc == FC - 1))
                def evac(q=q, ps2=ps2):
                    col = q * 512
                    for m in range(TT):
                        ev = evp.tile([P, 512], f16, name="ev")
                        nc.vector.scalar_tensor_tensor(
                            out=ev[:], in0=ps2[m][:], scalar=1.0,
                            in1=pk[m][:, col:col + 512],
                            op0=ALU.mult, op1=ALU.add)
                        nc.gpsimd.dma_start(
                            out=np_out[m * P:(m + 1) * P, col:col + 512],
                            in_=ev[:])
                if q == NQ - 1:
                    deferred.append(evac)
                else:
                    evac()

        flush_deferred()

    nc.compile()
    return nc


def prep_in_maps(inputs: dict) -> list[dict]:
    blocks = np.asarray(inputs["blocks"], np.float32).reshape(NB, TOK, D)
    pb = np.asarray(inputs["partial_block"], np.float32).reshape(TOK, D)
    w1 = np.asarray(inputs["ffn_w1"], np.float32)
    w2 = np.asarray(inputs["ffn_w2"], np.float32)
    w1h = np.ascontiguousarray(
        w1.reshape(DC, P, FC, P).transpose(2, 1, 0, 3)).astype(F16)
    w2h = np.ascontiguousarray(
        w2.reshape(FC, P, NQ, 512).transpose(2, 0, 1, 3)).astype(F16)
    pwh = (np.asarray(inputs["proj_w"], np.float32)
           * np.asarray(inputs["norm_scale"], np.float32)).astype(F16)
    in_maps = []
    for c in range(N_CORES):
        sl = slice(c * TPC, (c + 1) * TPC)
        vbc = np.concatenate([blocks[:, sl], pb[None, sl]],
                             axis=0).astype(F16)
        in_maps.append({"vb": vbc, "w1": w1h, "w2": w2h, "pw": pwh})
    return in_maps


_NC = None


def _get_nc():
    global _NC
    if _NC is None:
        _NC = build_nc()
    return _NC


def kernel(blocks, partial_block, proj_w, norm_scale, ffn_w1, ffn_w2):
    in_maps = prep_in_maps(dict(blocks=blocks, partial_block=partial_block,
                                proj_w=proj_w, norm_scale=norm_scale,
                                ffn_w1=ffn_w1, ffn_w2=ffn_w2))
    nc = _get_nc()
    res = run_bass_kernel_spmd(nc, in_maps, list(range(N_CORES)))
    h = np.concatenate([np.asarray(r["h_out"], dtype=np.float32)
                        for r in res.results], axis=0).reshape(B, T, D)
    npar = np.concatenate([np.asarray(r["np_out"], dtype=np.float32)
                           for r in res.results], axis=0).reshape(B, T, D)
    return h, npar
